# revision 14
# baseline (speedup 1.0000x reference)
"""Distributed brute-force KNN (IndexFlatL2, K=3) + mean of gathered pred values.

Strategy (data-parallel over the memory bank N, queries replicated):
  - Host sorts the memory rows by ||m||^2 and shards the sorted bank across
    the 8 cores (12500 rows each), transposed so the PE moving operand
    [K=d, N=n] streams straight from DRAM.
  - Device: c[b, n] = (2q).m_n via fp8e4m3 DoubleRow matmuls into fp32 PSUM
    (the PE runs at its fp8 roofline: ~208 ns per 500-column matmul).
    DVE tensor_reduce window-maxes each PSUM block (windows of 10 sorted
    rows) into a bf16 wmax tile; each group's finished wmax slice streams
    to DRAM while later groups compute. Input DMAs are split (per-query-
    chunk qT, per-k-pair memory tiles) so the first matmul starts as soon
    as the first slices land instead of after the whole first group.
  - Host: correct window maxes by the window-mean ||m||^2 (rows are
    msq-sorted, so within-window spread is ~0.04), pick the top WSEL
    windows per query (worst true-top-3 window rank observed on this
    dataset is 12, WSEL=96 is an 8x margin), exactly re-score their rows
    in fp32 BLAS, take the true top-3, gather pred_values, return the mean.
"""

import sys
import types

import ml_dtypes
import numpy as np

try:  # bass_utils' axon trace path imports this unconditionally when
    import antenv.axon_hooks  # noqa: F401  # BASS_TRACE is set; stub it if absent
except ImportError:
    _stub = types.ModuleType("antenv.axon_hooks")
    _stub.get_axon_ntff_profile_hook = lambda: None
    _stub.set_axon_ntff_profile_hook = lambda hook: None
    sys.modules["antenv.axon_hooks"] = _stub

import concourse.bacc as bacc
import concourse.mybir as mybir
import concourse.tile as tile
from concourse import bass_utils

B = 1024            # queries
D = 1024            # embedding dim
N = 100000          # memory rows
NCORES = 8
NS = N // NCORES    # 12500 memory rows per core
BLK = 500           # matmul free-dim tile (fits one PSUM bank in fp32)
NBLK = NS // BLK    # 25 blocks per core
GROUP_W = 5         # blocks fetched per DMA group (25 = 5 uniform groups)
KT = D // 128       # 8 contraction tiles
KSTEP = 2           # DoubleRow consumes k-tile pairs
BCH = B // 128      # 8 query chunks of 128
WND = 10            # window width for the DVE windowed max
NWIN = NS // WND    # 1250 windows per core
WPB = BLK // WND    # 50 windows per block
K = 3
WSEL = 96           # windows exactly re-scored on host per query

_CACHE = {}
LAST_RUN = None
LAST_TOP_IDX = None


def _build_program():
    nc = bacc.Bacc(
        "TRN2",
        target_bir_lowering=False,
        debug=False,
        enable_asserts=False,
        num_devices=NCORES,
    )
    f32 = mybir.dt.float32
    bf16 = mybir.dt.bfloat16
    mmdt = mybir.dt.float8e4

    mT = nc.dram_tensor("mT", [D, NS], mmdt, kind="ExternalInput").ap()
    qT = nc.dram_tensor("qT", [D, B], mmdt, kind="ExternalInput").ap()
    out_w = nc.dram_tensor("out_w", [128, BCH, NBLK, WPB], bf16, kind="ExternalOutput").ap()

    mT_r = mT.rearrange("(o p) n -> p o n", p=128)
    qT_r = qT.rearrange("(o p) b -> p o b", p=128)

    with tile.TileContext(nc) as tc:
        with (
            tc.tile_pool(name="const", bufs=1) as cpool,
            tc.tile_pool(name="mov", bufs=2) as movpool,
            tc.tile_pool(name="psum", bufs=8, space="PSUM") as pspool,
        ):
            # PE warm-up on zeroed scratch while input DMAs stream: keeps the
            # HAM activity window busy so the real matmuls all run at 2.4 GHz.
            scratch = cpool.tile([128, KSTEP, 512], mmdt, tag="warm")
            nc.vector.memset(scratch, 0)
            warm_ps = pspool.tile([128, BLK], f32, tag="mm", name="mm_ps")
            for _ in range(4):
                nc.tensor.matmul(
                    warm_ps,
                    lhsT=scratch[:, :, :128],
                    rhs=scratch[:, :, :BLK],
                    start=True,
                    stop=True,
                    perf_mode=mybir.MatmulPerfMode.DoubleRow,
                )

            qt_sb = cpool.tile([128, KT, B], mmdt, tag="qt")
            wmax = cpool.tile([128, BCH, NBLK, WPB], bf16, tag="wmax")

            # first group is small so early compute is cheap while the DMA
            # stream gets ahead of the consumption rate
            groups = [(0, 2), (2, 5), (7, 5), (12, 5), (17, 5), (22, 3)]
            for gi, (blk0, w) in enumerate(groups):
                n0 = blk0 * BLK
                wn = w * BLK
                mov = movpool.tile([128, KT, GROUP_W * BLK], mmdt, tag="mov")
                if gi == 0:
                    # per-k-pair DMAs so the first matmuls start early; the
                    # first k-pair goes before qt (it is the longer transfer)
                    nc.sync.dma_start(
                        mov[:, :KSTEP, :wn], mT_r[:, :KSTEP, n0 : n0 + wn]
                    )
                    nc.sync.dma_start(qt_sb[:, :, :128], qT_r[:, :, :128])
                    for kp in range(KSTEP, KT, KSTEP):
                        nc.sync.dma_start(
                            mov[:, kp : kp + KSTEP, :wn],
                            mT_r[:, kp : kp + KSTEP, n0 : n0 + wn],
                        )
                    nc.sync.dma_start(qt_sb[:, :, 128:], qT_r[:, :, 128:])
                else:
                    nc.sync.dma_start(mov[:, :, :wn], mT_r[:, :, n0 : n0 + wn])
                for bc in range(BCH):
                    psums = [
                        pspool.tile([128, BLK], f32, tag="mm", name="mm_ps")
                        for _ in range(w)
                    ]
                    for k in range(0, KT, KSTEP):
                        lhsT = qt_sb[:, k : k + KSTEP, bc * 128 : (bc + 1) * 128]
                        for j in range(w):
                            nc.tensor.matmul(
                                psums[j],
                                lhsT=lhsT,
                                rhs=mov[:, k : k + KSTEP, j * BLK : (j + 1) * BLK],
                                start=(k == 0),
                                stop=(k + KSTEP >= KT),
                                perf_mode=mybir.MatmulPerfMode.DoubleRow,
                            )
                    for j in range(w):
                        nc.vector.tensor_reduce(
                            wmax[:, bc, blk0 + j, :],
                            psums[j].rearrange("p (w t) -> p w t", t=WND),
                            axis=mybir.AxisListType.X,
                            op=mybir.AluOpType.max,
                            opt_input=False,
                        )
                    # this (group, query-chunk) slab is final -> stream out;
                    # for the very last slab, peel the final block off so the
                    # kernel-ending transfer is tiny
                    last_slab = gi == len(groups) - 1 and bc == BCH - 1
                    if last_slab and w > 1:
                        nc.sync.dma_start(
                            out_w[:, bc, blk0 : blk0 + w - 1],
                            wmax[:, bc, blk0 : blk0 + w - 1],
                        )
                        nc.sync.dma_start(
                            out_w[:, bc, blk0 + w - 1 : blk0 + w],
                            wmax[:, bc, blk0 + w - 1 : blk0 + w],
                        )
                    else:
                        nc.sync.dma_start(
                            out_w[:, bc, blk0 : blk0 + w],
                            wmax[:, bc, blk0 : blk0 + w],
                        )
    nc.compile()
    return nc


def kernel(h_query, memory_embeds, pred_values):
    global LAST_RUN, LAST_TOP_IDX
    q = np.ascontiguousarray(np.asarray(h_query, dtype=np.float32))
    m = np.ascontiguousarray(np.asarray(memory_embeds, dtype=np.float32))
    pv = np.asarray(pred_values, dtype=np.float32)

    msq_full = np.einsum("nd,nd->n", m, m)
    perm = np.argsort(msq_full, kind="stable")
    m_s = m[perm]                      # msq-sorted memory bank
    msq_s = msq_full[perm]

    qTs = (np.ascontiguousarray(q.T) * np.float32(2.0)).astype(ml_dtypes.float8_e4m3)
    mTs = np.ascontiguousarray(m_s.T).astype(ml_dtypes.float8_e4m3)
    msqw_all = msq_s.reshape(N // WND, WND).mean(axis=1).astype(np.float32)

    if "nc" not in _CACHE:
        _CACHE["nc"] = _build_program()
    nc = _CACHE["nc"]

    in_maps = []
    for c in range(NCORES):
        sl = slice(c * NS, (c + 1) * NS)
        in_maps.append(
            {
                "mT": np.ascontiguousarray(mTs[:, sl]),
                "qT": qTs,
            }
        )

    res = bass_utils.run_bass_kernel_spmd(nc, in_maps, core_ids=list(range(NCORES)))
    LAST_RUN = res
    results = res.results

    # out_w [128, BCH, NBLK, WPB] -> per-core window scores [B, NWIN]
    vals = np.concatenate(
        [
            np.ascontiguousarray(
                r["out_w"].astype(np.float32).transpose(1, 0, 2, 3)
            ).reshape(B, NWIN)
            for r in results
        ],
        axis=1,
    )  # [B, NCORES*NWIN]; global window w covers sorted rows [w*WND, +WND)
    vals -= msqw_all[None, :]

    # Phase 2: pick top-WSEL windows per query, exactly re-score their rows.
    sel = np.argpartition(-vals, WSEL, axis=1)[:, :WSEL]   # [B, WSEL]
    rows = sel[:, :, None] * WND + np.arange(WND)[None, None, :]
    cidx = rows.reshape(B, WSEL * WND)                     # sorted-space rows
    top_sorted = np.empty((B, K), np.int64)
    for b in range(B):
        mg = m_s.take(cidx[b], axis=0)                     # [WSEL*WND, D]
        s = 2.0 * (mg @ q[b]) - msq_s[cidx[b]]
        pick = np.argpartition(-s, K)[:K]
        top_sorted[b] = cidx[b][pick]
    top_idx = perm[top_sorted]                             # original row ids
    LAST_TOP_IDX = top_idx
    y = pv[top_idx].astype(np.float64).mean()
    return np.float32(y)
